# revision 26
# baseline (speedup 1.0000x reference)
"""MAMDense kernel for Trainium2 (8 NeuronCores, SPMD over row shards).

C[i,j] = max_k(x[i,k]*W[j,k]) + min_k(x[i,k]*W[j,k]) + bias[j]

v4 strategy (default): one fused custom DVE op does everything.
  - Shard rows M=2048 across 8 cores (256 rows each); W/bias replicated.
  - Per core, x rows are broadcast-replicated across all 128 SBUF
    partitions by DMA (DRAM read with partition-stride 0), fp16.
  - MAM_FUSED2X_ANT: a hand-assembled custom DVE op with a 2X_1PORT
    uop program (DveOpSpec.uops_2x + perf_max=1).  Each cycle it reads
    2 packed-fp16 W elements (port 0) and 2 packed-fp16 x elements
    (port 1), multiplies both pairs, and feeds in-pipe running-max and
    running-min scan accumulators = 2 products/cycle/lane, the DVE
    read-port bound.  A 3-state FSM (entry/steady/step on SUB_DIM_DONE)
    resets the scans at every row boundary of the [P, S, 768] access
    pattern, so one instruction covers S=16 rows.  Per cycle it writes
    the [fp16 running-min | fp16 running-max] pair; an overlapping
    page-stride -4 output AP keeps only each row's final pair.
  - Per (group, j-block) a tiny tensor op adds final min+max+bias into
    ct[j, row]; DMA ct -> HBM; host transposes/concats shards.
  - DVE is ~100% busy at its port bound; PE/ScalarE/GpSimd paths for
    product generation exist behind env knobs (MAM_G_GPS / MAM_G_F)
    but measured slower than the fused op's DVE floor, so the default
    runs everything through the fused op (~676us/iter vs 1020us for
    the v3 PE+PSUM-scan baseline).

v3 strategy (fallback, kept intact below):

v3 strategy (fp16 products, fp32 reduction; rel err ~1.6e-3):
  - Shard the flattened row dim M=2048 across 8 cores (256 rows each).
  - Per core, compute C^T [N, MC]: partitions = output cols j, free = rows i.
  - Per row i:
      * 6 diag tiles dg16[c] = diag(x[i, c*128:(c+1)*128]) in fp16
        (tensor_scalar of a fp16 identity by the per-partition x^T column).
      * PE: 36 matmuls p[j, k] = W[j,k]*x[i,k] into PSUM fp32
        (lhsT = W^T chunk [k,j] fp16 stationary, rhs = dg16[c]).
      * ScalarE copies the second k-half of p PSUM -> SBUF.
      * One custom DVE op (MAM_DUAL_MINMAX_ANT) streams the PSUM half and
        the SBUF half (2 products/cycle/lane) and computes
        running-max + running-min + bias with 4 in-pipe scan accumulators;
        a zero-stride out AP keeps only the final element, written directly
        to ct[b][:, i].  One DVE pass does BOTH the max and min chains.
  - DMA ct -> HBM; host transposes/concats shards.

The custom DVE op is registered at runtime into concourse.dve_ops (the
uop table is generated per-NEFF, no firmware change).  This walrus build
needs codegen_inst_isa_subclasses() run explicitly to fill InstISA bytes,
and accepts only ONE semaphore wait per instruction (post-pass splits
extra waits onto NoOps; the Tile drain is patched the same way).
"""

import os
import numpy as np

M_FULL, K, N, NCORES = 2048, 768, 768, 8
MC = M_FULL // NCORES
JB = N // 128
KC = K // 128
HALF = K // 2
FMAX = float(np.finfo(np.float32).max)

_STATE = {}
LAST_RUN_SECONDS = None

OP_NAME = "MAM_DUAL_MINMAX_ANT"


# --------------------------------------------------------------------------
# custom DVE op: dual min/max scan over two product streams, + bias
# --------------------------------------------------------------------------
def _register_mam_op():
    import concourse.dve_ops as dve_ops

    for o in dve_ops.OPS:
        if o.name == OP_NAME:
            return o

    from concourse.dve_spec import (
        Spec,
        Src0,
        Src1,
        C0,
        C1,
        AluOp,
        scan,
        maxx,
        minn,
        lower,
        _has_src1,
    )
    from concourse.dve_uop import DveOpSpec

    body = (
        maxx(scan(AluOp.MAX, Src0), scan(AluOp.MAX, Src1))
        + minn(scan(AluOp.MIN, Src0, init=C0), scan(AluOp.MIN, Src1, init=C0))
    ) + C1

    def ref(in0, in1, s0, s1, imm2):
        f32 = np.float32
        a0 = np.maximum.accumulate(in0.astype(f32), axis=-1)
        a1 = np.maximum.accumulate(in1.astype(f32), axis=-1)
        s0a = np.broadcast_to(np.asarray(s0, f32).reshape(-1, 1), in0.shape[:1] + (1,))
        m0 = np.minimum(np.minimum.accumulate(in0.astype(f32), axis=-1), s0a)
        m1 = np.minimum(np.minimum.accumulate(in1.astype(f32), axis=-1), s0a)
        s1a = np.broadcast_to(np.asarray(s1, f32).reshape(-1, 1), in0.shape[:1] + (1,))
        return (np.maximum(a0, a1) + np.minimum(m0, m1) + s1a).astype(f32)

    spec = Spec(body=body, reference=ref)
    row = dve_ops._CUSTOM_DVE_ROW_BASE + len(dve_ops.OPS)
    shas = {}
    for ver in ("v3", "v4"):
        try:
            uops = lower(spec, ver=ver)
            s = DveOpSpec(name=OP_NAME, opcode=row, uops=uops, rd1_en=_has_src1(spec))
            shas[ver] = s.sha(ver)
        except Exception:
            pass

    op = dve_ops.DveOp(OP_NAME, spec, subdim=False, uops_sha=shas)
    dve_ops.OPS.append(op)
    dve_ops.CUSTOM_DVE_SPECS[OP_NAME] = spec
    dve_ops._SUB_OPCODE_FOR_NAME[OP_NAME] = row
    return op


# --------------------------------------------------------------------------
# v4 raw custom DVE ops: hand-assembled 2x-mode segmented dual min/max scans
# --------------------------------------------------------------------------
SCAN_OP_NAME = "MAM_SCAN2X_ANT"
FUSED_OP_NAME = "MAM_FUSED2X_ANT"


def _register_raw_ops():
    """Register MAM_SCAN2X_ANT / MAM_FUSED2X_ANT with hand-built REGULAR and
    2X_1PORT uop programs + segmented (per-page) scan reset FSM."""
    from dataclasses import dataclass

    import concourse.dve_ops as dve_ops
    from concourse.dve_ops import DveOp
    from concourse.dve_spec import (
        AluOp,
        C0,
        Spec,
        Src0,
        Src1,
        maxx,
        minn,
        scan,
    )
    from concourse.dve_uop import (
        AluInp,
        DelayInp,
        DveOpSpec,
        InpSel,
        OutPath,
        OutSel,
        Trigger,
        UopConfig,
        UopDpConfig,
    )

    have = {o.name: o for o in dve_ops.OPS}
    if SCAN_OP_NAME in have and FUSED_OP_NAME in have:
        return have[SCAN_OP_NAME], have[FUSED_OP_NAME]

    PREV = AluInp.PREV_ALU_OUT
    CURR = AluInp.CURR_ALU_OUT
    D = [AluInp.PREV_DELAY_0, AluInp.PREV_DELAY_1, AluInp.PREV_DELAY_2,
         AluInp.PREV_DELAY_3, AluInp.PREV_DELAY_4, AluInp.PREV_DELAY_5]

    def states(mk_dp):
        sts = []
        for i, reset in ((0, True), (1, False), (2, True)):
            u = UopConfig()
            u.datapath_config = mk_dp(reset)
            u.require_inp0 = 1
            u.require_inp1 = 1
            u.enable_output(OutSel.ALU_OUT, OutPath.WR0_LO)
            if i == 1:
                u.trigger = (Trigger.SRC_TENSOR_DONE, Trigger.SUB_DIM_DONE,
                             Trigger.NONE)
                u.next_uop = (0, 2, 0)
            else:
                u.trigger = (Trigger.SRC_TENSOR_DONE, Trigger.SUB_DIM_DONE,
                             Trigger.COUNT)
                u.next_uop = (0, 2, 1)
                u.repeat_count = 1
            sts.append(u)
        return sts

    def scan_dp_2x(reset):
        bs = [UopDpConfig() for _ in range(8)]
        bs[0].enable_alu(AluOp.MAX, D[0], D[1]).pass_through_delay(0, 1, 2, 3)
        bs[1].enable_alu(AluOp.MAX, PREV, D[2]).pass_through_delay(0, 1, 2, 3)
        bs[2].enable_alu(AluOp.MAX, PREV, D[3]).pass_through_delay(0, 1, 2, 3)
        bs[3].enable_alu(AluOp.MAX if not reset else AluOp.BYPASS,
                         CURR if not reset else PREV,
                         PREV).pass_through_delay(0, 1, 2, 3)
        bs[4].enable_alu(AluOp.MIN, D[0], D[1]).pass_through_delay(2, 3)
        bs[4].enable_delay_from_src(DelayInp.PREV_ALU_OUT, 4)
        bs[5].enable_alu(AluOp.MIN, PREV, D[2]).pass_through_delay(3, 4)
        bs[6].enable_alu(AluOp.MIN, PREV, D[3]).pass_through_delay(4)
        bs[7].enable_alu(AluOp.MIN if not reset else AluOp.BYPASS,
                         CURR if not reset else PREV,
                         PREV).pass_through_delay(4)
        return bs

    def scan_dp_1x(reset):
        bs = [UopDpConfig() for _ in range(8)]
        bs[0].enable_alu(AluOp.MAX, D[0], D[1]).pass_through_delay(0, 1)
        bs[1].enable_alu(AluOp.MAX if not reset else AluOp.BYPASS,
                         CURR if not reset else PREV,
                         PREV).pass_through_delay(0, 1)
        bs[2].enable_alu(AluOp.MIN, D[0], D[1])
        bs[2].enable_delay_from_src(DelayInp.PREV_ALU_OUT, 2)
        bs[3].enable_alu(AluOp.MIN if not reset else AluOp.BYPASS,
                         CURR if not reset else PREV,
                         PREV).pass_through_delay(2)
        for b in range(4, 8):
            bs[b].pass_through_alu().pass_through_delay(2)
        return bs

    def fused_dp_2x(reset):
        bs = [UopDpConfig() for _ in range(8)]
        bs[0].enable_alu(AluOp.MULTIPLY, D[0], D[2]).pass_through_delay(1, 3)
        bs[1].enable_alu(AluOp.MULTIPLY, D[1], D[3])
        bs[1].enable_delay_from_src(DelayInp.PREV_ALU_OUT, 0)  # m0
        bs[2].enable_alu(AluOp.MAX, PREV, D[0]).pass_through_delay(0)
        bs[2].enable_delay_from_src(DelayInp.PREV_ALU_OUT, 1)  # m1
        bs[3].enable_alu(AluOp.MAX if not reset else AluOp.BYPASS,
                         CURR if not reset else PREV,
                         PREV).pass_through_delay(0, 1)
        bs[4].enable_alu(AluOp.MIN, D[0], D[1])
        bs[4].enable_delay_from_src(DelayInp.PREV_ALU_OUT, 2)  # running max
        bs[5].enable_alu(AluOp.MIN if not reset else AluOp.BYPASS,
                         CURR if not reset else PREV,
                         PREV).pass_through_delay(2)
        for b in range(6, 8):
            bs[b].pass_through_alu().pass_through_delay(2)
        return bs

    def fused_dp_1x(reset):
        bs = [UopDpConfig() for _ in range(8)]
        bs[0].enable_alu(AluOp.MULTIPLY, D[0], D[1])
        bs[1].enable_alu(AluOp.MAX if not reset else AluOp.BYPASS,
                         CURR if not reset else PREV, PREV)
        bs[1].enable_delay_from_src(DelayInp.PREV_ALU_OUT, 2)  # product
        bs[2].enable_alu(AluOp.MIN if not reset else AluOp.BYPASS,
                         CURR if not reset else D[2], D[2])
        bs[2].enable_delay_from_src(DelayInp.PREV_ALU_OUT, 3)  # running max
        for b in range(3, 8):
            bs[b].pass_through_alu().pass_through_delay(3)
        return bs

    def mk_scan():
        u1 = states(scan_dp_1x)
        for u in u1:
            u.enable_input(InpSel.SRC_0, 1)
            u.enable_input(InpSel.SRC_1, 2)
            u.enable_output(OutSel.DELAY_2, OutPath.WR0_HI)
        u2 = states(scan_dp_2x)
        for u in u2:
            u.enable_input(InpSel.SRC_0, 1)
            u.enable_input(InpSel.SRC_0_HI, 2)
            u.enable_input(InpSel.SRC_1, 3)
            u.enable_input(InpSel.SRC_1_HI, 4)
            u.enable_output(OutSel.DELAY_4, OutPath.WR0_HI)
        return u1, u2

    def mk_fused():
        u1 = states(fused_dp_1x)
        for u in u1:
            u.enable_input(InpSel.SRC_0, 1)
            u.enable_input(InpSel.SRC_1, 2)
            u.enable_output(OutSel.DELAY_3, OutPath.WR0_HI)
        u2 = states(fused_dp_2x)
        for u in u2:
            u.enable_input(InpSel.SRC_0, 1)
            u.enable_input(InpSel.SRC_0_HI, 2)
            u.enable_input(InpSel.SRC_1, 3)
            u.enable_input(InpSel.SRC_1_HI, 4)
            u.enable_output(OutSel.DELAY_2, OutPath.WR0_HI)
        return u1, u2

    @dataclass(frozen=True)
    class RawDveOp(DveOp):
        raw: "DveOpSpec | None" = None

        def compile(self, ver):
            assert ver == "v3" and self.raw is not None
            return self.raw

    body = maxx(scan(AluOp.MAX, Src0), scan(AluOp.MAX, Src1)) + minn(
        scan(AluOp.MIN, Src0, init=C0), scan(AluOp.MIN, Src1, init=C0)
    )
    nominal = Spec(body=body, reference=lambda *a: None)

    out_ops = []
    for name, mk in ((SCAN_OP_NAME, mk_scan), (FUSED_OP_NAME, mk_fused)):
        if name in have:
            out_ops.append(have[name])
            continue
        u1, u2 = mk()
        row = dve_ops._CUSTOM_DVE_ROW_BASE + len(dve_ops.OPS)
        raw = DveOpSpec(name=name, opcode=row, uops=u1, uops_2x=u2,
                        perf_max=1, rd1_en=True)
        raw.validate("v3")
        op = RawDveOp(name=name, spec=nominal, subdim=True, uops_sha={},
                      raw=raw)
        dve_ops.OPS.append(op)
        dve_ops.CUSTOM_DVE_SPECS[name] = op.spec
        dve_ops._SUB_OPCODE_FOR_NAME[name] = row
        out_ops.append(op)
    return out_ops[0], out_ops[1]


# --------------------------------------------------------------------------
# v4 builder: three product pipelines feeding packed-fp16 dual min/max scans
# --------------------------------------------------------------------------
def _build_nc_v4(loop_n=1):
    import contextlib

    import concourse.bass as bass
    import concourse.mybir as mybir
    import concourse.tile as tile
    from concourse.vector_clock import ScopedClock

    scan_op, fused_op = _register_raw_ops()
    _patch_tile_drain(tile, mybir, ScopedClock)

    F32 = mybir.dt.float32
    F16 = mybir.dt.float16

    S = int(os.environ.get("MAM_S", "8"))
    G = MC // S
    Q = S // 4  # row quads per group
    OBW = 4 * (S - 1) + 768 + 4
    n_gps = int(os.environ.get("MAM_G_GPS", "0"))
    n_f = int(os.environ.get("MAM_G_F", "-1"))
    if n_f < 0:
        # default: ~10/32 of groups through the PE+ScalarE delivery path
        # (deferred scans), rest through the fused op
        n_f = G - n_gps - max(0, round(G * 10 / 32))
    n_pe = G - n_gps - n_f
    assert n_pe >= 0
    epi_eng = os.environ.get("MAM_EPI", "gpstt")
    diag_scal = int(os.environ.get("MAM_DIAG_SCAL", "6"))
    LOOK = int(os.environ.get("MAM_LOOK", "1"))
    ps_bufs = int(os.environ.get("MAM_PS_BUFS", "4"))
    prod_bufs = int(os.environ.get("MAM_PROD_BUFS", "8"))
    xbc_bufs = int(os.environ.get("MAM_XBC_BUFS", "2"))
    scan_delay = int(os.environ.get("MAM_SCAN_DELAY", "3"))
    ob_bufs = int(os.environ.get("MAM_OB_BUFS", "6"))

    # interleaved path assignment (p = PE+copy, g = gpsimd gen, f = fused)
    assigns = []
    err = {"p": 0.0, "g": 0.0, "f": 0.0}
    cnt = {"p": n_pe, "g": n_gps, "f": n_f}
    for i in range(G):
        for k in err:
            err[k] += cnt[k] / G
        best = max(err, key=lambda k: err[k])
        err[best] -= 1.0
        assigns.append(best)

    nc = bass.Bass("TRN2", debug=False)
    wt16 = nc.dram_tensor("wt16", [K, N], F16, kind="ExternalInput")
    wr16 = nc.dram_tensor("wr16", [N, K], F16, kind="ExternalInput")
    xt32 = nc.dram_tensor("xt32", [K, MC], F32, kind="ExternalInput")
    xs16 = nc.dram_tensor("xs16", [MC, K], F16, kind="ExternalInput")
    id16 = nc.dram_tensor("id16", [128, 128], F16, kind="ExternalInput")
    bias = nc.dram_tensor("bias_in", [N], F32, kind="ExternalInput")
    ct = nc.dram_tensor("ct", [N, MC], F32, kind="ExternalOutput")

    def ap3(base, inner_off, mid, npages, ninner):
        return bass.AP(
            tensor=base.tensor,
            offset=base.offset + inner_off,
            ap=[list(base.ap[0]), [mid, npages], [1, ninner]],
        )

    def ap2(base, off, stride, n):
        return bass.AP(tensor=base.tensor, offset=base.offset + off,
                       ap=[list(base.ap[0]), [stride, n]])

    with tile.TileContext(nc) as tc:
        with tc.tile_pool(name="singles", bufs=1) as singles, tc.tile_pool(
            name="dgpool", bufs=LOOK + 1
        ) as dgpool, tc.tile_pool(
            name="pspool", bufs=ps_bufs, space="PSUM"
        ) as pspool, tc.tile_pool(
            name="prodpool", bufs=prod_bufs
        ) as prodpool, tc.tile_pool(
            name="xbcpool", bufs=xbc_bufs
        ) as xbcpool, tc.tile_pool(name="obpool", bufs=ob_bufs) as obpool:
            # ---- resident setup (outside the timing loop) ----
            w_sb = [
                [singles.tile([128, 128], F16, tag=f"w{c}_{b}", name=f"w{c}_{b}")
                 for b in range(JB)]
                for c in range(KC)
            ]
            for c in range(KC):
                for b in range(JB):
                    nc.sync.dma_start(
                        out=w_sb[c][b][:],
                        in_=wt16.ap()[c * 128:(c + 1) * 128,
                                      b * 128:(b + 1) * 128],
                    )
            wb_sb = [singles.tile([128, K], F16, tag=f"wb{b}", name=f"wb{b}")
                     for b in range(JB)]
            for b in range(JB):
                nc.sync.dma_start(out=wb_sb[b][:],
                                  in_=wr16.ap()[b * 128:(b + 1) * 128, :])
            xt_sb = [singles.tile([128, MC], F32, tag=f"xt{c}", name=f"xt{c}")
                     for c in range(KC)]
            for c in range(KC):
                nc.sync.dma_start(out=xt_sb[c][:],
                                  in_=xt32.ap()[c * 128:(c + 1) * 128, :])
            id_sb = singles.tile([128, 128], F16, tag="id16")
            nc.sync.dma_start(out=id_sb[:], in_=id16.ap())
            bias_sb = singles.tile([128, JB], F32, tag="bias")
            nc.sync.dma_start(out=bias_sb[:],
                              in_=bias.ap().rearrange("(b p) -> p b", p=128))
            ct_sb = [singles.tile([128, MC], F32, tag=f"ct{b}", name=f"ct{b}")
                     for b in range(JB)]

            loop_cm = tc.For_i(0, loop_n, 1) if loop_n > 1 else (
                contextlib.nullcontext())
            loop_cm.__enter__() if hasattr(loop_cm, "__enter__") else None

            pending_dg = {}
            pending_xbc = {}

            def pre(g):
                kind = assigns[g]
                if kind == "p":
                    dgs = []
                    for q in range(Q):
                        for c in range(KC):
                            dg = dgpool.tile([128, 512], F16,
                                             tag=f"dg{q}_{c}", name="dg")
                            for r in range(4):
                                row = g * S + q * 4 + r
                                dst = dg[:, r * 128:(r + 1) * 128]
                                if c < diag_scal:
                                    nc.scalar.activation(
                                        out=dst, in_=id_sb[:],
                                        func=mybir.ActivationFunctionType.Copy,
                                        scale=xt_sb[c][:, row:row + 1])
                                else:
                                    nc.vector.tensor_scalar(
                                        out=dst, in0=id_sb[:],
                                        scalar1=xt_sb[c][:, row:row + 1],
                                        scalar2=None,
                                        op0=mybir.AluOpType.mult)
                            dgs.append(dg)
                    pending_dg[g] = dgs
                else:
                    xb = xbcpool.tile([128, S * K], F16, tag="xbc", name="xbc")
                    sap = xs16.ap()
                    bap = bass.AP(tensor=sap.tensor,
                                  offset=sap.offset + g * S * K,
                                  ap=[[0, 128], [1, S * K]])
                    nc.sync.dma_start(out=xb[:], in_=bap)
                    pending_xbc[g] = xb

            def epilogue(g, b, ob, anchor):
                mins = ap2(ob[:], anchor + 4 * (S - 1), -4, S)
                maxs = ap2(ob[:], anchor + 1 + 4 * (S - 1), -4, S)
                if epi_eng == "gpstt":
                    # min+max on GpSimd; bias folded in at end of iteration
                    nc.gpsimd.tensor_tensor(
                        out=ct_sb[b][:, g * S:(g + 1) * S],
                        in0=mins, in1=maxs, op=mybir.AluOpType.add)
                    return
                eng = nc.gpsimd if epi_eng == "gps" else nc.vector
                eng.scalar_tensor_tensor(
                    out=ct_sb[b][:, g * S:(g + 1) * S],
                    in0=mins,
                    scalar=bias_sb[:, b:b + 1],
                    in1=maxs,
                    op0=mybir.AluOpType.add,
                    op1=mybir.AluOpType.add,
                )

            def emit_scan(g, b, prod):
                ob = obpool.tile([128, OBW], F16, tag="ob", name="ob")
                i0 = ap3(prod[:], 0, K, S, 384)
                i1 = ap3(prod[:], 384, K, S, 384)
                o0 = ap3(ob[:], 4 * (S - 1), -4, S, 384)
                ins = nc.vector._custom_dve(scan_op, out=o0, in0=i0, in1=i1,
                                            s0=0.0, s1=0.0)
                ins.ins.perf_max = 1
                epilogue(g, b, ob, 382)

            pend_scans = []  # (emitted_at_group_idx, closure)

            def flush_scans(now_idx, force=False):
                while pend_scans and (
                    force or now_idx - pend_scans[0][0] >= scan_delay
                ):
                    pend_scans.pop(0)[1]()

            def main(g):
                kind = assigns[g]
                if kind == "p":
                    dgs = pending_dg.pop(g)
                    for b in range(JB):
                        prod = prodpool.tile([128, S * K], F16, tag="prod",
                                             name="prod")
                        for c in range(KC):
                            for qp in range(Q // 2):
                                pt = pspool.tile([128, 1024], F32, tag="ps",
                                                 name="ps")
                                for qq in range(2):
                                    q = qp * 2 + qq
                                    nc.tensor.matmul(
                                        out=pt[:, qq * 512:(qq + 1) * 512],
                                        lhsT=w_sb[c][b][:],
                                        rhs=dgs[q * KC + c][:],
                                        start=True, stop=True)
                                dst = ap3(prod[:], qp * 8 * K + c * 128,
                                          K, 8, 128)
                                nc.scalar.copy(out=dst, in_=pt[:])
                        pend_scans.append(
                            (g, lambda g=g, b=b, prod=prod: emit_scan(g, b, prod)))
                elif kind == "g":
                    xb = pending_xbc[g]
                    for b in range(JB):
                        prod = prodpool.tile([128, S * K], F16, tag="prod",
                                             name="prod")
                        i0 = ap3(wb_sb[b][:], 0, 0, S, K)
                        i1 = ap3(xb[:], 0, K, S, K)
                        nc.gpsimd.tensor_tensor(out=prod[:], in0=i0, in1=i1,
                                                op=mybir.AluOpType.mult)
                        pend_scans.append(
                            (g, lambda g=g, b=b, prod=prod: emit_scan(g, b, prod)))
                    pending_xbc.pop(g)
                else:
                    xb = pending_xbc[g]
                    for b in range(JB):
                        ob = obpool.tile([128, OBW], F16, tag="ob", name="ob")
                        f0 = ap3(wb_sb[b][:], 0, 0, S, K)
                        f1 = ap3(xb[:], 0, K, S, K)
                        of = ap3(ob[:], 4 * (S - 1), -4, S, K)
                        ins = nc.vector._custom_dve(fused_op, out=of, in0=f0,
                                                    in1=f1, s0=0.0, s1=0.0)
                        ins.ins.perf_max = 1
                        epilogue(g, b, ob, 766)
                    pending_xbc.pop(g)

            for g in range(G):
                if g == 0:
                    for j in range(min(LOOK + 1, G)):
                        pre(j)
                elif g + LOOK < G:
                    pre(g + LOOK)
                main(g)
                flush_scans(g)
            flush_scans(G, force=True)

            # ---- bias (gpstt epilogue defers it) + writeback ----
            if epi_eng == "gpstt":
                for b in range(JB):
                    nc.vector.tensor_scalar(
                        out=ct_sb[b][:], in0=ct_sb[b][:],
                        scalar1=bias_sb[:, b:b + 1], scalar2=None,
                        op0=mybir.AluOpType.add)
            ct_re = ct.ap().rearrange("(b p) m -> b p m", p=128)
            for b in range(JB):
                nc.sync.dma_start(out=ct_re[b], in_=ct_sb[b][:])
            loop_cm.__exit__(None, None, None)

    mybir.codegen_inst_isa_subclasses(nc)
    _split_sem_waits(nc, mybir)
    return nc


# --------------------------------------------------------------------------
# walrus single-sem-wait workarounds (carried over from the v1 baseline)
# --------------------------------------------------------------------------
def _patch_tile_drain(tile, mybir, ScopedClock, maxw=1):
    if getattr(tile.TileContext, "_mam_drain_patched", False):
        return

    def _pd(self, tick_clock, wait_clock):
        nc = self.nc
        drain_inst = nc.sync.drain()
        wait_clock.add_sem_waits(
            drain_inst.ins, ScopedClock({None: tick_clock.global_clock})
        )
        si = drain_inst.ins.sync_info
        waits = list(si.on_wait) if si is not None else []
        if len(waits) > maxw:
            si.on_wait = waits[:maxw]
            for i in range(maxw, len(waits), maxw):
                nop = nc.sync.nop(nofuse=True, hint="waitsplit")
                nop.ins.sync_info = mybir.SyncInfo(
                    on_wait=list(waits[i : i + maxw]), on_update=[]
                )
        nc.all_engine_barrier()
        popped = nc._tile_sem_poison_stack.pop()
        assert popped is self._sem_poison
        nc.clear_and_free_semaphores(list(self.sems.allocated().values()))
        nc.all_engine_barrier()

    tile.TileContext._drain_and_barrier = _pd
    tile.TileContext._mam_drain_patched = True


def _split_sem_waits(nc, mybir, maxw=1):
    n = 0
    for f in nc.m.functions:
        for blk in f.blocks:
            insts = blk.instructions
            i = 0
            while i < len(insts):
                inst = insts[i]
                si = inst.sync_info
                if si is not None and len(si.on_wait) > maxw:
                    waits = list(si.on_wait)
                    si.on_wait = waits[:maxw]
                    rest = waits[maxw:]
                    for j in range(0, len(rest), maxw):
                        n += 1
                        nop = mybir.InstNoOp(
                            name=f"I-wsplit-{n}-{inst.name}",
                            engine=inst.engine,
                            ins=[],
                            outs=[],
                            sync_info=mybir.SyncInfo(
                                on_wait=list(rest[j : j + maxw]), on_update=[]
                            ),
                        )
                        nc.register_instruction(nop)
                        insts.insert(i, nop)
                        i += 1
                i += 1
    return n


# --------------------------------------------------------------------------
# v3 builder
# --------------------------------------------------------------------------
def _build_nc_v3(loop_n=1, diag_engine="vector"):
    import contextlib
    import concourse.bass as bass
    import concourse.tile as tile
    import concourse.mybir as mybir
    from concourse.vector_clock import ScopedClock

    mam_op = _register_mam_op()
    _patch_tile_drain(tile, mybir, ScopedClock)

    F32 = mybir.dt.float32
    F16 = mybir.dt.float16

    nc = bass.Bass("TRN2", debug=False)
    wt16 = nc.dram_tensor("wt16", [K, N], F16, kind="ExternalInput")  # weight.T fp16
    xt32 = nc.dram_tensor("xt32", [K, MC], F32, kind="ExternalInput")  # x-shard^T fp32
    id16 = nc.dram_tensor("id16", [128, 128], F16, kind="ExternalInput")
    bias = nc.dram_tensor("bias_in", [N], F32, kind="ExternalInput")
    ct = nc.dram_tensor("ct", [N, MC], F32, kind="ExternalOutput")  # C^T shard

    pair_default = os.environ.get("MAM_PAIR", "0") == "1"
    split_default = os.environ.get("MAM_SPLIT_PS", "1") == "1"
    ps_bufs = int(os.environ.get("MAM_PS_BUFS", "2" if pair_default else "4"))
    with tile.TileContext(nc) as tc:
        loop_cm = tc.For_i(0, loop_n, 1) if loop_n > 1 else contextlib.nullcontext()
        with tc.tile_pool(name="singles", bufs=1) as singles, tc.tile_pool(
            name="dgpool", bufs=int(os.environ.get("MAM_DG_BUFS", "24"))
        ) as dgpool, tc.tile_pool(name="pspool", bufs=ps_bufs, space="PSUM") as pspool, tc.tile_pool(
            name="sbpool", bufs=int(os.environ.get("MAM_SB_BUFS", "12"))
        ) as sbpool:
            # -------- setup (outside the timing loop: weights stay resident) --------
            w_sb = [
                [
                    singles.tile([128, 128], F16, tag=f"w{c}_{b}", name=f"w{c}_{b}")
                    for b in range(JB)
                ]
                for c in range(KC)
            ]
            for c in range(KC):
                for b in range(JB):
                    nc.sync.dma_start(
                        out=w_sb[c][b][:],
                        in_=wt16.ap()[c * 128 : (c + 1) * 128, b * 128 : (b + 1) * 128],
                    )
            xt_sb = [
                singles.tile([128, MC], F32, tag=f"xt{c}", name=f"xt{c}")
                for c in range(KC)
            ]
            for c in range(KC):
                nc.sync.dma_start(
                    out=xt_sb[c][:], in_=xt32.ap()[c * 128 : (c + 1) * 128, :]
                )
            id_sb = singles.tile([128, 128], F16, tag="id16")
            nc.sync.dma_start(out=id_sb[:], in_=id16.ap())
            bias_sb = singles.tile([128, JB], F32, tag="bias")
            nc.sync.dma_start(
                out=bias_sb[:], in_=bias.ap().rearrange("(b p) -> p b", p=128)
            )
            ct_sb = [
                singles.tile([128, MC], F32, tag=f"ct{b}", name=f"ct{b}")
                for b in range(JB)
            ]

            # how many of the 6 per-row diag builds go to ScalarE (rest on DVE)
            n_scal = int(os.environ.get("MAM_DIAG_SCAL", "2"))
            loop_ctx_entered = True
            loop_cm.__enter__() if hasattr(loop_cm, "__enter__") else None
            ablate = os.environ.get("MAM_ABLATE", "none")

            # -------- main loop over rows --------
            # Diag builds are emitted D rows ahead so they sit in front of
            # the row's custom ops in the DVE FIFO; otherwise PE stalls at
            # every row boundary waiting for the next row's diags.
            D = int(os.environ.get("MAM_DIAG_AHEAD", "3"))

            def build_diags(i):
                dgs = [None] * KC
                for c in range(KC):
                    dg = dgpool.tile([128, 128], F16, tag=f"dg{c}")
                    if c < n_scal:
                        nc.scalar.activation(
                            out=dg[:],
                            in_=id_sb[:],
                            func=mybir.ActivationFunctionType.Copy,
                            scale=xt_sb[c][:, i : i + 1],
                        )
                    else:
                        nc.vector.tensor_scalar(
                            out=dg[:],
                            in0=id_sb[:],
                            scalar1=xt_sb[c][:, i : i + 1],
                            scalar2=None,
                            op0=mybir.AluOpType.mult,
                        )
                    dgs[c] = dg
                return dgs

            pair_mode = os.environ.get("MAM_PAIR", "0") == "1"

            if not pair_mode:
                split_ps = os.environ.get("MAM_SPLIT_PS", "1") == "1"
                pending = {}
                for i in range(min(D, MC)):
                    pending[i] = build_diags(i)
                for i in range(MC):
                    if i + D < MC:
                        pending[i + D] = build_diags(i + D)
                    dgs = pending.pop(i)
                    for b in range(JB):
                        if split_ps:
                            # two bank-sized PSUM tiles per group -> 8 bufs ->
                            # PE runs 4 groups ahead of the DVE scans
                            p_a = pspool.tile([128, 512], F32, tag="ppa")
                            p_b = pspool.tile([128, 512], F32, tag="ppb")
                            for c in (3, 4, 5):
                                nc.tensor.matmul(
                                    out=p_b[:, (c - 3) * 128 : (c - 2) * 128],
                                    lhsT=w_sb[c][b][:],
                                    rhs=dgs[c][:],
                                    start=True,
                                    stop=True,
                                )
                            p_sb = sbpool.tile([128, HALF], F32, tag="psb")
                            nc.scalar.copy(out=p_sb[:], in_=p_b[:, :HALF])
                            for c in (0, 1, 2):
                                nc.tensor.matmul(
                                    out=p_a[:, c * 128 : (c + 1) * 128],
                                    lhsT=w_sb[c][b][:],
                                    rhs=dgs[c][:],
                                    start=True,
                                    stop=True,
                                )
                            src0 = p_a[:, :HALF]
                        else:
                            p_ps = pspool.tile([128, K], F32, tag="pp")
                            for c in (3, 4, 5, 0, 1, 2):
                                nc.tensor.matmul(
                                    out=p_ps[:, c * 128 : (c + 1) * 128],
                                    lhsT=w_sb[c][b][:],
                                    rhs=dgs[c][:],
                                    start=True,
                                    stop=True,
                                )
                            p_sb = sbpool.tile([128, HALF], F32, tag="psb")
                            nc.scalar.copy(out=p_sb[:], in_=p_ps[:, HALF:])
                            src0 = p_ps[:, :HALF]
                        base = ct_sb[b][:, i : i + 1]
                        out_z = bass.AP(
                            tensor=base.tensor,
                            offset=base.offset,
                            ap=[list(base.ap[0]), [0, HALF]],
                        )
                        nc.vector._custom_dve(
                            mam_op,
                            out=out_z,
                            in0=src0,
                            in1=p_sb[:],
                            s0=FMAX,
                            s1=bias_sb[:, b : b + 1],
                        )
            else:
                # Row-pair mode: one LDWEIGHTS serves a 256-col matmul for two
                # rows ([diag_i | diag_{i+1}]), one ScalarE copy moves both
                # rows' second k-half, and the custom op picks its row's
                # products with 2-free-dim APs.  Bias can't ride the op here
                # (rank-3 src1 drops the C1 AP slot) so it's applied at the
                # end on the ct tiles.
                def build_diag_pair(ip):
                    i = 2 * ip
                    dgp = [None] * KC
                    for c in range(KC):
                        dg = dgpool.tile([128, 256], F16, tag=f"dgp{c}")
                        for r in range(2):
                            if c < n_scal:
                                nc.scalar.activation(
                                    out=dg[:, r * 128 : (r + 1) * 128],
                                    in_=id_sb[:],
                                    func=mybir.ActivationFunctionType.Copy,
                                    scale=xt_sb[c][:, i + r : i + r + 1],
                                )
                            else:
                                nc.vector.tensor_scalar(
                                    out=dg[:, r * 128 : (r + 1) * 128],
                                    in0=id_sb[:],
                                    scalar1=xt_sb[c][:, i + r : i + r + 1],
                                    scalar2=None,
                                    op0=mybir.AluOpType.mult,
                                )
                        dgp[c] = dg
                    return dgp

                NPAIR = MC // 2
                pending = {}
                for ip in range(min(D, NPAIR)):
                    pending[ip] = build_diag_pair(ip)
                for ip in range(NPAIR):
                    if ip + D < NPAIR:
                        pending[ip + D] = build_diag_pair(ip + D)
                    dgp = pending.pop(ip)
                    i = 2 * ip
                    for b in range(JB):
                        p_ps = pspool.tile([128, 2 * K], F32, tag="ppp")
                        for c in (3, 4, 5, 0, 1, 2):
                            nc.tensor.matmul(
                                out=p_ps[:, c * 256 : (c + 1) * 256],
                                lhsT=w_sb[c][b][:],
                                rhs=dgp[c][:],
                                start=True,
                                stop=True,
                            )
                        p_sb = sbpool.tile([128, K], F32, tag="psbp")
                        nc.scalar.copy(out=p_sb[:], in_=p_ps[:, 3 * 256 :])
                        for r in range(2):
                            base0 = p_ps[:, r * 128 : r * 128 + 1]
                            src0 = bass.AP(
                                tensor=base0.tensor,
                                offset=base0.offset,
                                ap=[list(base0.ap[0]), [256, 3], [1, 128]],
                            )
                            base1 = p_sb[:, r * 128 : r * 128 + 1]
                            src1 = bass.AP(
                                tensor=base1.tensor,
                                offset=base1.offset,
                                ap=[list(base1.ap[0]), [256, 3], [1, 128]],
                            )
                            basec = ct_sb[b][:, i + r : i + r + 1]
                            out_z = bass.AP(
                                tensor=basec.tensor,
                                offset=basec.offset,
                                ap=[list(basec.ap[0]), [0, HALF]],
                            )
                            nc.vector._custom_dve(
                                mam_op,
                                out=out_z,
                                in0=src0,
                                in1=src1,
                                s0=FMAX,
                                s1=0.0,
                            )
                for b in range(JB):
                    nc.vector.tensor_scalar(
                        out=ct_sb[b][:],
                        in0=ct_sb[b][:],
                        scalar1=bias_sb[:, b : b + 1],
                        scalar2=None,
                        op0=mybir.AluOpType.add,
                    )
            # -------- writeback --------
            ct_re = ct.ap().rearrange("(b p) m -> b p m", p=128)
            for b in range(JB):
                nc.sync.dma_start(out=ct_re[b], in_=ct_sb[b][:])
            loop_cm.__exit__(None, None, None)

    mybir.codegen_inst_isa_subclasses(nc)
    _split_sem_waits(nc, mybir)
    return nc


# --------------------------------------------------------------------------
# v1 builder (fallback, from the baseline)
# --------------------------------------------------------------------------
def _build_nc_v1(loop_n=1):
    import contextlib
    import concourse.bass as bass
    import concourse.tile as tile
    import concourse.mybir as mybir
    from concourse.vector_clock import ScopedClock

    _patch_tile_drain(tile, mybir, ScopedClock)

    DT = mybir.dt.float32
    nc = bass.Bass("TRN2", debug=False)
    xs = nc.dram_tensor("xs", [MC, K], DT, kind="ExternalInput")
    wt = nc.dram_tensor("wt", [K, N], DT, kind="ExternalInput")
    bias = nc.dram_tensor("bias_in", [N], DT, kind="ExternalInput")
    out = nc.dram_tensor("out", [MC, N], DT, kind="ExternalOutput")
    with tile.TileContext(nc) as tc:
        loop_cm = tc.For_i(0, loop_n, 1) if loop_n > 1 else contextlib.nullcontext()
        with loop_cm, tc.tile_pool(name="singles", bufs=1) as singles, tc.tile_pool(
            name="bpool", bufs=32
        ) as bpool:
            x_re = xs.ap().rearrange("(t p) k -> t p k", p=128)
            o_re = out.ap().rearrange("(t p) n -> t p n", p=128)
            ntiles = MC // 128
            x_sb, amax, amin = [], [], []
            for t in range(ntiles):
                xt = singles.tile([128, K], DT, tag=f"x{t}")
                nc.sync.dma_start(out=xt[:], in_=x_re[t])
                x_sb.append(xt)
                mx = singles.tile([128, N], DT, tag=f"amax{t}")
                mn = singles.tile([128, N], DT, tag=f"amin{t}")
                nc.vector.memset(mx[:], -FMAX)
                nc.vector.memset(mn[:], FMAX)
                amax.append(mx)
                amin.append(mn)
            biasb = singles.tile([128, N], DT, tag="biasb")
            bap = bias.ap()
            nc.sync.dma_start(
                out=biasb[:],
                in_=bass.AP(
                    tensor=bap.tensor, offset=bap.offset, ap=[[0, 128], list(bap.ap[0])]
                ),
            )
            for k in range(K):
                bt = bpool.tile([128, N], DT, tag="b")
                row = wt.ap()[k : k + 1, :]
                nc.sync.dma_start(
                    out=bt[:],
                    in_=bass.AP(
                        tensor=row.tensor,
                        offset=row.offset,
                        ap=[[0, 128], list(row.ap[1])],
                    ),
                )
                for t in range(ntiles):
                    nc.vector.scalar_tensor_tensor(
                        out=amax[t][:],
                        in0=bt[:],
                        scalar=x_sb[t][:, k : k + 1],
                        in1=amax[t][:],
                        op0=mybir.AluOpType.mult,
                        op1=mybir.AluOpType.max,
                    )
                    nc.vector.scalar_tensor_tensor(
                        out=amin[t][:],
                        in0=bt[:],
                        scalar=x_sb[t][:, k : k + 1],
                        in1=amin[t][:],
                        op0=mybir.AluOpType.mult,
                        op1=mybir.AluOpType.min,
                    )
            for t in range(ntiles):
                nc.vector.tensor_tensor(
                    out=amax[t][:],
                    in0=amax[t][:],
                    in1=amin[t][:],
                    op=mybir.AluOpType.add,
                )
                nc.vector.tensor_tensor(
                    out=amax[t][:],
                    in0=amax[t][:],
                    in1=biasb[:],
                    op=mybir.AluOpType.add,
                )
                nc.sync.dma_start(out=o_re[t], in_=amax[t][:])
    _split_sem_waits(nc, mybir)
    return nc


# --------------------------------------------------------------------------
# runner (bass2jax shard_map over 8 cores, from the baseline)
# --------------------------------------------------------------------------
def _make_runner(nc, n_cores=NCORES):
    import jax
    from jax.sharding import Mesh, PartitionSpec
    from jax.experimental.shard_map import shard_map
    import concourse.mybir as mybir
    from concourse import bass2jax

    bass2jax.install_neuronx_cc_hook()

    partition_name = nc.partition_id_tensor.name if nc.partition_id_tensor else None
    in_names, out_names, out_avals, zero_shapes = [], [], [], []
    for alloc in nc.m.functions[0].allocations:
        if not isinstance(alloc, mybir.MemoryLocationSet):
            continue
        name = alloc.memorylocations[0].name
        if alloc.kind == "ExternalInput":
            if name != partition_name:
                in_names.append(name)
        elif alloc.kind == "ExternalOutput":
            shape = tuple(alloc.tensor_shape)
            dtype = mybir.dt.np(alloc.dtype)
            out_names.append(name)
            out_avals.append(jax.core.ShapedArray(shape, dtype))
            zero_shapes.append((shape, dtype))
    n_params = len(in_names)
    n_outs = len(out_avals)
    in_names_all = list(in_names) + list(out_names)
    if partition_name is not None:
        in_names_all.append(partition_name)

    def _body(*args):
        operands = list(args)
        if partition_name is not None:
            operands.append(bass2jax.partition_id_tensor())
        outs = bass2jax._bass_exec_p.bind(
            *operands,
            out_avals=tuple(out_avals),
            in_names=tuple(in_names_all),
            out_names=tuple(out_names),
            lowering_input_output_aliases=(),
            sim_require_finite=True,
            sim_require_nnan=True,
            nc=nc,
        )
        return tuple(outs)

    devices = jax.devices()[:n_cores]
    mesh = Mesh(np.asarray(devices), ("core",))
    in_specs = (PartitionSpec("core"),) * (n_params + n_outs)
    out_specs = (PartitionSpec("core"),) * n_outs
    sharded = jax.jit(
        shard_map(
            _body, mesh=mesh, in_specs=in_specs, out_specs=out_specs, check_rep=False
        ),
        keep_unused=True,
    )

    def run(in_maps):
        global LAST_RUN_SECONDS
        import time as _time

        per_core = [[np.asarray(m[nm]) for nm in in_names] for m in in_maps]
        concat_in = [
            np.concatenate([per_core[c][i] for c in range(n_cores)], axis=0)
            for i in range(n_params)
        ]
        concat_zeros = [
            np.zeros((n_cores * s[0], *s[1:]), d) for (s, d) in zero_shapes
        ]
        t0 = _time.time()
        out_arrs = sharded(*concat_in, *concat_zeros)
        out_np = [np.asarray(a) for a in out_arrs]
        LAST_RUN_SECONDS = _time.time() - t0
        return [
            {
                nm: out_np[i].reshape(n_cores, *out_avals[i].shape)[c]
                for i, nm in enumerate(out_names)
            }
            for c in range(n_cores)
        ]

    run.sharded = sharded
    run.in_names = in_names
    run.zero_shapes = zero_shapes
    run.out_names = out_names
    run.out_avals = out_avals
    run.mesh = mesh
    return run


def _fallback_runner(nc):
    from concourse.bass_utils import run_bass_kernel_spmd

    def run(in_maps):
        res = run_bass_kernel_spmd(nc, in_maps, core_ids=list(range(NCORES)))
        return res.results

    return run


IMPL = os.environ.get("MAM_IMPL", "v4")
DIAG_ENGINE = os.environ.get("MAM_DIAG_ENGINE", "vector")


def _build(impl, loop_n=1):
    if impl == "v4":
        return _build_nc_v4(loop_n=loop_n)
    if impl == "v3":
        return _build_nc_v3(loop_n=loop_n, diag_engine=DIAG_ENGINE)
    return _build_nc_v1(loop_n=loop_n)


def _get_runner():
    if "runner" not in _STATE:
        chain = {"v4": ("v4", "v3", "v1"), "v3": ("v3", "v1"), "v1": ("v1",)}
        for impl in chain.get(IMPL, ("v4", "v3", "v1")):
            try:
                nc = _build(impl)
                _STATE["runner"] = _make_runner(nc)
                _STATE["impl"] = impl
                break
            except Exception:
                if impl == "v1" or os.environ.get("MAM_STRICT"):
                    raise
                import traceback

                traceback.print_exc()
    return _STATE["runner"], _STATE["impl"]


def _run_with_retry(run, in_maps, impl):
    try:
        return run(in_maps)
    except Exception:
        _STATE.pop("runner", None)
        try:
            nc = _build(impl)
            run2 = _fallback_runner(nc)
            return run2(in_maps)
        except Exception:
            if impl in ("v3", "v4"):
                raise _FallbackToV1()
            raise


class _FallbackToV1(Exception):
    pass


def _in_maps_v4(xf, W, b):
    wt16 = np.ascontiguousarray(W.T.astype(np.float16))
    wr16 = np.ascontiguousarray(W.astype(np.float16))
    ident16 = np.eye(128, dtype=np.float16)
    b32 = np.ascontiguousarray(b.astype(np.float32))
    maps = []
    for c in range(NCORES):
        sh = xf[c * MC:(c + 1) * MC]
        maps.append({
            "wt16": wt16,
            "wr16": wr16,
            "xt32": np.ascontiguousarray(sh.T.astype(np.float32)),
            "xs16": np.ascontiguousarray(sh.astype(np.float16)),
            "id16": ident16,
            "bias_in": b32,
        })
    return maps


def _in_maps_v3(xf, W, b):
    wt16 = np.ascontiguousarray(W.T.astype(np.float16))
    ident16 = np.eye(128, dtype=np.float16)
    b32 = np.ascontiguousarray(b.astype(np.float32))
    return [
        {
            "wt16": wt16,
            "xt32": np.ascontiguousarray(xf[c * MC : (c + 1) * MC].T.astype(np.float32)),
            "id16": ident16,
            "bias_in": b32,
        }
        for c in range(NCORES)
    ]


def _in_maps_v1(xf, W, b):
    wt = np.ascontiguousarray(W.T)
    b32 = np.ascontiguousarray(b.astype(np.float32))
    return [
        {"xs": xf[c * MC : (c + 1) * MC], "wt": wt, "bias_in": b32}
        for c in range(NCORES)
    ]


def kernel(x, weight, bias):
    x = np.ascontiguousarray(np.asarray(x, dtype=np.float32))
    W = np.ascontiguousarray(np.asarray(weight, dtype=np.float32))
    b = np.ascontiguousarray(np.asarray(bias, dtype=np.float32))
    run, impl = _get_runner()
    xf = x.reshape(-1, K)
    if impl in ("v3", "v4"):
        try:
            mk = _in_maps_v4 if impl == "v4" else _in_maps_v3
            in_maps = mk(xf, W, b)
            outs = _run_with_retry(run, in_maps, impl)
            C = np.concatenate([o["ct"].T for o in outs], axis=0)
        except _FallbackToV1:
            _STATE.clear()
            nc = _build_nc_v1()
            run = _fallback_runner(nc)
            in_maps = _in_maps_v1(xf, W, b)
            outs = run(in_maps)
            C = np.concatenate([o["out"] for o in outs], axis=0)
    else:
        in_maps = _in_maps_v1(xf, W, b)
        outs = _run_with_retry(run, in_maps, impl)
        C = np.concatenate([o["out"] for o in outs], axis=0)
    return np.ascontiguousarray(
        C.reshape(x.shape[:-1] + (W.shape[0],)), dtype=np.float32
    )

